# revision 8
# baseline (speedup 1.0000x reference)
"""AttentionSumReader on 8 trn2 cores — batch data-parallel.

Per core (8 examples): scores = doc@q on DVE (mul + 3D reduce), maskless
softmax (no max-subtract: |scores| <~ 60 so exp fits fp32), segment-sum
over 600 entity ids via factored one-hot matmul (id = hi*25+lo,
seg[24,25] = (exp*onehot_hi)^T @ onehot_lo accumulated over 32 s-chunks
on the PE), then log(seg/denom + eps).

Host precomputes (cheap, integer/bool transforms): hi/lo/mask in the
[p, c] layout (s = c*128 + p), q replicated across partitions, small
constant tables.
"""

import os
import sys

sys.path.insert(0, "/opt/trn_rl_repo")

import numpy as np

B, S, E = 64, 4096, 128
NUM_ENTITIES = 600
HI, LO = 24, 25  # 600 = 24*25, id = hi*25 + lo
NCORES = 8
BP = B // NCORES  # examples per core
NCHUNK = S // 128  # 32 s-chunks of 128
GRP = 8  # chunks per DMA/compute group
NGRP = NCHUNK // GRP
LOG_EPS = 1e-12

LAST_RESULTS = None  # BassKernelResults stash for test.py


def _build_bass():
    import concourse.bass as bass
    import concourse.tile as tile
    from concourse import bacc, mybir

    f32 = mybir.dt.float32

    nc = bacc.Bacc(
        "TRN2",
        target_bir_lowering=False,
        debug=False,
        enable_asserts=False,
        num_devices=NCORES,
    )

    doc_d = nc.dram_tensor("doc", (BP, S, E), f32, kind="ExternalInput").ap()
    qrep_d = nc.dram_tensor("qrep", (BP, 128, E), f32, kind="ExternalInput").ap()
    hi_d = nc.dram_tensor("hi", (BP, 128, NCHUNK), f32, kind="ExternalInput").ap()
    lo_d = nc.dram_tensor("lo", (BP, 128, NCHUNK), f32, kind="ExternalInput").ap()
    msk_d = nc.dram_tensor("msk", (BP, 128, NCHUNK), f32, kind="ExternalInput").ap()
    hrange_d = nc.dram_tensor("hrange", (128, HI), f32, kind="ExternalInput").ap()
    lorange_d = nc.dram_tensor("lorange", (128, LO), f32, kind="ExternalInput").ap()
    ones_d = nc.dram_tensor("ones24", (128, HI), f32, kind="ExternalInput").ap()
    eps_d = nc.dram_tensor("epsc", (HI, 1), f32, kind="ExternalInput").ap()
    out_d = nc.dram_tensor("out", (BP, NUM_ENTITIES), f32, kind="ExternalOutput").ap()

    with tile.TileContext(nc) as tc:
        with (
            tc.tile_pool(name="const", bufs=1) as constp,
            tc.tile_pool(name="docp", bufs=3) as docp,
            tc.tile_pool(name="prodp", bufs=2) as prodp,
            tc.tile_pool(name="ohp", bufs=2) as ohp,
            tc.tile_pool(name="qp", bufs=2) as qp,
            tc.tile_pool(name="smallp", bufs=4) as smallp,
            tc.tile_pool(name="psum", bufs=2, space="PSUM") as psump,
        ):
            hrange_t = constp.tile([128, HI], f32)
            nc.sync.dma_start(hrange_t[:], hrange_d)
            lorange_t = constp.tile([128, LO], f32)
            nc.sync.dma_start(lorange_t[:], lorange_d)
            ones_t = constp.tile([128, HI], f32)
            nc.sync.dma_start(ones_t[:], ones_d)
            eps_t = constp.tile([HI, 1], f32)
            nc.sync.dma_start(eps_t[:], eps_d)

            for b in range(BP):
                doc_b = doc_d[b].rearrange("(c p) e -> p c e", p=128)

                qrep_t = qp.tile([128, E], f32, tag="qrep")
                nc.sync.dma_start(qrep_t[:], qrep_d[b])

                scores_t = smallp.tile([128, NCHUNK], f32, tag="scores")
                for g in range(NGRP):
                    doc_t = docp.tile([128, GRP, E], f32, tag="doc")
                    nc.sync.dma_start(doc_t[:], doc_b[:, g * GRP : (g + 1) * GRP, :])
                    prod_t = prodp.tile([128, GRP, E], f32, tag="prod")
                    nc.vector.tensor_mul(
                        prod_t[:],
                        doc_t[:],
                        qrep_t[:].unsqueeze(1).broadcast_to((128, GRP, E)),
                    )
                    nc.vector.reduce_sum(
                        scores_t[:, g * GRP : (g + 1) * GRP],
                        prod_t[:],
                        axis=mybir.AxisListType.X,
                    )

                # maskless exp; zero out padded positions after
                exps_t = smallp.tile([128, NCHUNK], f32, tag="exps")
                nc.scalar.activation(
                    exps_t[:], scores_t[:], mybir.ActivationFunctionType.Exp
                )
                msk_t = smallp.tile([128, NCHUNK], f32, tag="msk")
                nc.sync.dma_start(msk_t[:], msk_d[b])
                expm_t = smallp.tile([128, NCHUNK], f32, tag="expm")
                nc.vector.tensor_mul(expm_t[:], exps_t[:], msk_t[:])
                colsum_t = smallp.tile([128, 1], f32, tag="colsum")
                nc.vector.reduce_sum(colsum_t[:], expm_t[:], axis=mybir.AxisListType.X)

                # denom replicated onto HI partitions via ones-matmul
                denom_p = psump.tile([HI, 1], f32, tag="denom")
                nc.tensor.matmul(
                    denom_p[:], ones_t[:], colsum_t[:], start=True, stop=True
                )

                hi_t = smallp.tile([128, NCHUNK], f32, tag="hi")
                nc.sync.dma_start(hi_t[:], hi_d[b])
                lo_t = smallp.tile([128, NCHUNK], f32, tag="lo")
                nc.sync.dma_start(lo_t[:], lo_d[b])

                a_oh = ohp.tile([128, NCHUNK, HI], f32, tag="a_oh")
                nc.vector.tensor_tensor(
                    a_oh[:],
                    hi_t[:].unsqueeze(2).broadcast_to((128, NCHUNK, HI)),
                    hrange_t[:].unsqueeze(1).broadcast_to((128, NCHUNK, HI)),
                    op=mybir.AluOpType.is_equal,
                )
                nc.vector.tensor_mul(
                    a_oh[:],
                    a_oh[:],
                    expm_t[:].unsqueeze(2).broadcast_to((128, NCHUNK, HI)),
                )
                b_oh = ohp.tile([128, NCHUNK, LO], f32, tag="b_oh")
                nc.vector.tensor_tensor(
                    b_oh[:],
                    lo_t[:].unsqueeze(2).broadcast_to((128, NCHUNK, LO)),
                    lorange_t[:].unsqueeze(1).broadcast_to((128, NCHUNK, LO)),
                    op=mybir.AluOpType.is_equal,
                )

                seg_p = psump.tile([HI, LO], f32, tag="seg")
                for c in range(NCHUNK):
                    nc.tensor.matmul(
                        seg_p[:],
                        a_oh[:, c, :],
                        b_oh[:, c, :],
                        start=(c == 0),
                        stop=(c == NCHUNK - 1),
                    )

                inv_t = smallp.tile([HI, 1], f32, tag="inv")
                nc.vector.reciprocal(inv_t[:], denom_p[:])
                sums_t = smallp.tile([HI, LO], f32, tag="sums")
                nc.vector.tensor_scalar_mul(sums_t[:], seg_p[:], inv_t[:])
                logit_t = smallp.tile([HI, LO], f32, tag="logit")
                nc.scalar.activation(
                    logit_t[:],
                    sums_t[:],
                    mybir.ActivationFunctionType.Ln,
                    bias=eps_t[:],
                )
                nc.sync.dma_start(
                    out_d[b].rearrange("(h l) -> h l", l=LO), logit_t[:]
                )

    nc.compile()
    return nc


def build_in_maps(document_emb, query_emb, document_ids, sequence_length):
    doc = np.ascontiguousarray(np.asarray(document_emb, dtype=np.float32))
    q = np.asarray(query_emb, dtype=np.float32)
    ids = np.asarray(document_ids, dtype=np.int32)
    slen = np.asarray(sequence_length, dtype=np.int32)

    def pc_layout(a):  # [B, S] -> [B, 128, 32] with s = c*128 + p
        return np.ascontiguousarray(
            a.reshape(B, NCHUNK, 128).transpose(0, 2, 1).astype(np.float32)
        )

    hi = pc_layout(ids // LO)
    lo = pc_layout(ids % LO)
    msk = pc_layout((np.arange(S)[None, :] < slen[:, None]).astype(np.float32))
    qrep = np.ascontiguousarray(
        np.broadcast_to(q[:, None, :], (B, 128, E)).astype(np.float32)
    )
    hrange = np.ascontiguousarray(
        np.broadcast_to(np.arange(HI, dtype=np.float32)[None, :], (128, HI))
    )
    lorange = np.ascontiguousarray(
        np.broadcast_to(np.arange(LO, dtype=np.float32)[None, :], (128, LO))
    )
    ones24 = np.ones((128, HI), dtype=np.float32)

    in_maps = []
    for i in range(NCORES):
        sl = slice(i * BP, (i + 1) * BP)
        in_maps.append(
            {
                "doc": doc[sl],
                "qrep": qrep[sl],
                "hi": hi[sl],
                "lo": lo[sl],
                "msk": msk[sl],
                "hrange": hrange,
                "lorange": lorange,
                "ones24": ones24,
                "epsc": np.full((HI, 1), LOG_EPS, dtype=np.float32),
            }
        )
    return in_maps


def kernel(document_emb, query_emb, document_ids, sequence_length):
    global LAST_RESULTS
    from concourse.bass_utils import run_bass_kernel_spmd

    in_maps = build_in_maps(document_emb, query_emb, document_ids, sequence_length)
    nc = _build_bass()

    trace = os.environ.get("KERNEL_TRACE", "0") == "1"
    res = run_bass_kernel_spmd(
        nc, in_maps, core_ids=list(range(NCORES)), trace=trace
    )
    LAST_RESULTS = res
    out = np.concatenate([res.results[i]["out"] for i in range(NCORES)], axis=0)
    return np.ascontiguousarray(out.astype(np.float32))


# revision 10
# speedup vs baseline: 866.4935x; 866.4935x over previous
"""AttentionSumReader on 8 trn2 cores — batch data-parallel.

Per core (8 examples): scores = doc@q on DVE (mul + 3D reduce), maskless
softmax (no max-subtract: |scores| <~ 60 so exp fits fp32), segment-sum
over 600 entity ids via factored one-hot matmul (id = hi*25+lo,
seg[24,25] = (exp*onehot_hi)^T @ onehot_lo accumulated over 32 s-chunks
on the PE), then log(seg/denom + eps).

Host precomputes (cheap, integer/bool transforms): hi/lo/mask in the
[p, c] layout (s = c*128 + p), q replicated across partitions, small
constant tables.
"""

import os
import sys

sys.path.insert(0, "/opt/trn_rl_repo")

import numpy as np

B, S, E = 64, 4096, 128
NUM_ENTITIES = 600
HI, LO = 24, 25  # 600 = 24*25, id = hi*25 + lo
NCORES = 8
BP = B // NCORES  # examples per core
NCHUNK = S // 128  # 32 s-chunks of 128
GRP = 8  # chunks per DMA/compute group
NGRP = NCHUNK // GRP
LOG_EPS = 1e-12

LAST_RESULTS = None  # BassKernelResults stash for test.py


def _build_bass(reps=1):
    import concourse.bass as bass
    import concourse.tile as tile
    from concourse import bacc, mybir

    f32 = mybir.dt.float32

    nc = bacc.Bacc(
        "TRN2",
        target_bir_lowering=False,
        debug=False,
        enable_asserts=False,
        num_devices=NCORES,
    )

    doc_d = nc.dram_tensor("doc", (BP, S, E), f32, kind="ExternalInput").ap()
    qrep_d = nc.dram_tensor("qrep", (BP, 128, E), f32, kind="ExternalInput").ap()
    hi_d = nc.dram_tensor("hi", (BP, 128, NCHUNK), f32, kind="ExternalInput").ap()
    lo_d = nc.dram_tensor("lo", (BP, 128, NCHUNK), f32, kind="ExternalInput").ap()
    msk_d = nc.dram_tensor("msk", (BP, 128, NCHUNK), f32, kind="ExternalInput").ap()
    hrange_d = nc.dram_tensor("hrange", (128, HI), f32, kind="ExternalInput").ap()
    lorange_d = nc.dram_tensor("lorange", (128, LO), f32, kind="ExternalInput").ap()
    ones_d = nc.dram_tensor("ones24", (128, HI), f32, kind="ExternalInput").ap()
    eps_d = nc.dram_tensor("epsc", (HI, 1), f32, kind="ExternalInput").ap()
    out_d = nc.dram_tensor("out", (BP, NUM_ENTITIES), f32, kind="ExternalOutput").ap()

    with tile.TileContext(nc) as tc:
        with (
            tc.tile_pool(name="const", bufs=1) as constp,
            tc.tile_pool(name="docp", bufs=3) as docp,
            tc.tile_pool(name="prodp", bufs=2) as prodp,
            tc.tile_pool(name="ohp", bufs=2) as ohp,
            tc.tile_pool(name="qp", bufs=2) as qp,
            tc.tile_pool(name="smallp", bufs=4) as smallp,
            tc.tile_pool(name="psum", bufs=2, space="PSUM") as psump,
        ):
            hrange_t = constp.tile([128, HI], f32)
            nc.sync.dma_start(hrange_t[:], hrange_d)
            lorange_t = constp.tile([128, LO], f32)
            nc.sync.dma_start(lorange_t[:], lorange_d)
            ones_t = constp.tile([128, HI], f32)
            nc.sync.dma_start(ones_t[:], ones_d)
            eps_t = constp.tile([HI, 1], f32)
            nc.sync.dma_start(eps_t[:], eps_d)

            for b in [bb for _ in range(reps) for bb in range(BP)]:
                doc_b = doc_d[b].rearrange("(c p) e -> p c e", p=128)

                qrep_t = qp.tile([128, E], f32, tag="qrep")
                nc.sync.dma_start(qrep_t[:], qrep_d[b])

                scores_t = smallp.tile([128, NCHUNK], f32, tag="scores")
                for g in range(NGRP):
                    doc_t = docp.tile([128, GRP, E], f32, tag="doc")
                    nc.sync.dma_start(doc_t[:], doc_b[:, g * GRP : (g + 1) * GRP, :])
                    prod_t = prodp.tile([128, GRP, E], f32, tag="prod")
                    nc.vector.tensor_mul(
                        prod_t[:],
                        doc_t[:],
                        qrep_t[:].unsqueeze(1).broadcast_to((128, GRP, E)),
                    )
                    nc.vector.reduce_sum(
                        scores_t[:, g * GRP : (g + 1) * GRP],
                        prod_t[:],
                        axis=mybir.AxisListType.X,
                    )

                # maskless exp; zero out padded positions after
                exps_t = smallp.tile([128, NCHUNK], f32, tag="exps")
                nc.scalar.activation(
                    exps_t[:], scores_t[:], mybir.ActivationFunctionType.Exp
                )
                msk_t = smallp.tile([128, NCHUNK], f32, tag="msk")
                nc.sync.dma_start(msk_t[:], msk_d[b])
                expm_t = smallp.tile([128, NCHUNK], f32, tag="expm")
                nc.vector.tensor_mul(expm_t[:], exps_t[:], msk_t[:])
                colsum_t = smallp.tile([128, 1], f32, tag="colsum")
                nc.vector.reduce_sum(colsum_t[:], expm_t[:], axis=mybir.AxisListType.X)

                # denom replicated onto HI partitions via ones-matmul
                denom_p = psump.tile([HI, 1], f32, tag="denom")
                nc.tensor.matmul(
                    denom_p[:], ones_t[:], colsum_t[:], start=True, stop=True
                )

                hi_t = smallp.tile([128, NCHUNK], f32, tag="hi")
                nc.sync.dma_start(hi_t[:], hi_d[b])
                lo_t = smallp.tile([128, NCHUNK], f32, tag="lo")
                nc.sync.dma_start(lo_t[:], lo_d[b])

                a_oh = ohp.tile([128, NCHUNK, HI], f32, tag="a_oh")
                nc.vector.tensor_tensor(
                    a_oh[:],
                    hi_t[:].unsqueeze(2).broadcast_to((128, NCHUNK, HI)),
                    hrange_t[:].unsqueeze(1).broadcast_to((128, NCHUNK, HI)),
                    op=mybir.AluOpType.is_equal,
                )
                nc.vector.tensor_mul(
                    a_oh[:],
                    a_oh[:],
                    expm_t[:].unsqueeze(2).broadcast_to((128, NCHUNK, HI)),
                )
                b_oh = ohp.tile([128, NCHUNK, LO], f32, tag="b_oh")
                nc.vector.tensor_tensor(
                    b_oh[:],
                    lo_t[:].unsqueeze(2).broadcast_to((128, NCHUNK, LO)),
                    lorange_t[:].unsqueeze(1).broadcast_to((128, NCHUNK, LO)),
                    op=mybir.AluOpType.is_equal,
                )

                seg_p = psump.tile([HI, LO], f32, tag="seg")
                for c in range(NCHUNK):
                    nc.tensor.matmul(
                        seg_p[:],
                        a_oh[:, c, :],
                        b_oh[:, c, :],
                        start=(c == 0),
                        stop=(c == NCHUNK - 1),
                    )

                inv_t = smallp.tile([HI, 1], f32, tag="inv")
                nc.vector.reciprocal(inv_t[:], denom_p[:])
                sums_t = smallp.tile([HI, LO], f32, tag="sums")
                nc.vector.tensor_scalar_mul(sums_t[:], seg_p[:], inv_t[:])
                logit_t = smallp.tile([HI, LO], f32, tag="logit")
                nc.scalar.activation(
                    logit_t[:],
                    sums_t[:],
                    mybir.ActivationFunctionType.Ln,
                    bias=eps_t[:],
                )
                nc.sync.dma_start(
                    out_d[b].rearrange("(h l) -> h l", l=LO), logit_t[:]
                )

    nc.compile()
    return nc


def build_in_maps(document_emb, query_emb, document_ids, sequence_length):
    doc = np.ascontiguousarray(np.asarray(document_emb, dtype=np.float32))
    q = np.asarray(query_emb, dtype=np.float32)
    ids = np.asarray(document_ids, dtype=np.int32)
    slen = np.asarray(sequence_length, dtype=np.int32)

    def pc_layout(a):  # [B, S] -> [B, 128, 32] with s = c*128 + p
        return np.ascontiguousarray(
            a.reshape(B, NCHUNK, 128).transpose(0, 2, 1).astype(np.float32)
        )

    hi = pc_layout(ids // LO)
    lo = pc_layout(ids % LO)
    msk = pc_layout((np.arange(S)[None, :] < slen[:, None]).astype(np.float32))
    qrep = np.ascontiguousarray(
        np.broadcast_to(q[:, None, :], (B, 128, E)).astype(np.float32)
    )
    hrange = np.ascontiguousarray(
        np.broadcast_to(np.arange(HI, dtype=np.float32)[None, :], (128, HI))
    )
    lorange = np.ascontiguousarray(
        np.broadcast_to(np.arange(LO, dtype=np.float32)[None, :], (128, LO))
    )
    ones24 = np.ones((128, HI), dtype=np.float32)

    in_maps = []
    for i in range(NCORES):
        sl = slice(i * BP, (i + 1) * BP)
        in_maps.append(
            {
                "doc": doc[sl],
                "qrep": qrep[sl],
                "hi": hi[sl],
                "lo": lo[sl],
                "msk": msk[sl],
                "hrange": hrange,
                "lorange": lorange,
                "ones24": ones24,
                "epsc": np.full((HI, 1), LOG_EPS, dtype=np.float32),
            }
        )
    return in_maps


def kernel(document_emb, query_emb, document_ids, sequence_length):
    global LAST_RESULTS
    from concourse.bass_utils import run_bass_kernel_spmd

    in_maps = build_in_maps(document_emb, query_emb, document_ids, sequence_length)
    nc = _build_bass()

    trace = os.environ.get("KERNEL_TRACE", "0") == "1"
    res = run_bass_kernel_spmd(
        nc, in_maps, core_ids=list(range(NCORES)), trace=trace
    )
    LAST_RESULTS = res
    out = np.concatenate([res.results[i]["out"] for i in range(NCORES)], axis=0)
    return np.ascontiguousarray(out.astype(np.float32))
